# Initial kernel scaffold
#
# Multi-head attention kernel for Trainium2 (Bass/Tile), 8-core SPMD.
#
# Problem: B=4, S=2048, D=1024, H=16 heads, d_k=64 (fp32).
#
# Sharding: core c = (batch b, head-group g) with b = c//2, g = c%2.
# Each core computes 8 heads of one batch entirely on-device and emits the
# partial final projection (out_heads @ Wo_slice^T) over the full model dim.
# Host sums the two partial outputs per batch (the "all-reduce" of the
# tensor-parallel Wo) and adds the linear bias terms.
#
# Dataflow (per core) avoids every on-device transpose:
#   - host feeds x^T [D, S] so the contraction dim (d) is on partitions
#   - Q^T, K^T [e, s] computed directly (e on partitions)
#   - V [s, e] computed naturally (s on partitions), pre-scaled by the key
#     mask, with the mask itself appended as a 65th column per head so the
#     attention-V matmul also produces the softmax denominators (row 64).
#   - scores computed transposed S^T[k, q] = K^T.T-chunks @ Q^T, two heads
#     packed per PE pass via 64-row tile_position groups.
#   - exp on ScalarE straight out of PSUM in [128, 1536] batches
#   - attn@V via lhsT = [V*mask | mask] (M=65), accumulated over 16 k-chunks
#   - softmax normalization: reciprocal of row 64, gpsimd partition
#     broadcast, one DVE multiply per head.
#   - final^T[e, q] = Wo^T-chunks @ O^T accumulated over the 4 local d-chunks.
#
# Biases: bq/bk added on device (per-partition adds folded into the PSUM
# eviction). bv and bo are linear post-softmax terms: since softmax rows sum
# to one, (attn@V + bv)@Wo^T + bo == attn@V@Wo^T + (bv@Wo^T + bo), which the
# host adds to the gathered output.

from contextlib import ExitStack

import numpy as np

import concourse.bass as bass  # noqa: F401  (AP types come via handles)
import concourse.tile as tile
from concourse import bacc, mybir
from concourse.bass_utils import run_bass_kernel_spmd

P = 128
S = 2048          # sequence length
D = 1024          # model dim
E = 512           # per-core head dims (8 heads x 64)
NH = 8            # heads per core
NDCH = D // P     # 8 contraction chunks for projections
NST = S // P      # 16 s-tiles (key chunks)
NSC = 4           # s-chunks of 512
NET = E // P      # 4 e-tiles of the local head dims
NHP = NH // 2     # 4 head pairs
NKC = NST         # 16 key chunks of 128
NQC = 4           # query chunks of 512
QCW = S // NQC    # 512
VW = 65           # V columns per head incl. mask column

F32 = mybir.dt.float32
F32R = mybir.dt.float32r
AF = mybir.ActivationFunctionType

EXP_GRP = 3       # scores tiles per exp instruction (3 PSUM banks)


def _mm(nc, out, lhsT, rhs, start, stop):
    nc.tensor.matmul(
        out,
        lhsT,
        rhs,
        start=start,
        stop=stop,
    )


def _build_program():
    nc = bacc.Bacc(
        "TRN2",
        debug=False,
        target_bir_lowering=False,
        enable_partition_id=False,
    )

    xT = nc.dram_tensor("xT", [D, S], F32R, kind="ExternalInput").ap()
    wqT = nc.dram_tensor("wqT", [D, E], F32R, kind="ExternalInput").ap()
    wkT = nc.dram_tensor("wkT", [D, E], F32R, kind="ExternalInput").ap()
    wvT = nc.dram_tensor("wvT", [D, E], F32R, kind="ExternalInput").ap()
    woT = nc.dram_tensor("woT", [E, D], F32R, kind="ExternalInput").ap()
    bq_t = nc.dram_tensor("bq_t", [P, NET], F32, kind="ExternalInput").ap()
    bk_t = nc.dram_tensor("bk_t", [P, NET], F32, kind="ExternalInput").ap()
    mk_t = nc.dram_tensor("mk_t", [P, NST], F32, kind="ExternalInput").ap()
    mask8 = nc.dram_tensor("mask8", [NST, P, NH], F32R, kind="ExternalInput").ap()
    fT = nc.dram_tensor("fT", [D, S], F32, kind="ExternalOutput").ap()

    with tile.TileContext(nc) as tc, ExitStack() as ctx:
        pers = ctx.enter_context(tc.tile_pool(name="pers", bufs=1))

        KT = [pers.tile([P, S], F32R, name=f"KT{j}", tag=f"KT{j}") for j in range(NET)]
        QT = [pers.tile([P, S], F32R, name=f"QT{j}", tag=f"QT{j}") for j in range(NET)]
        Vg = [
            pers.tile([P, NH * VW], F32R, name=f"Vg{t}", tag=f"Vg{t}")
            for t in range(NST)
        ]
        bq_sb = pers.tile([P, NET], F32, name="bq_sb", tag="bq_sb")
        bk_sb = pers.tile([P, NET], F32, name="bk_sb", tag="bk_sb")
        mk_sb = pers.tile([P, NST], F32, name="mk_sb", tag="mk_sb")
        nc.gpsimd.dma_start(bq_sb[:], bq_t)
        nc.gpsimd.dma_start(bk_sb[:], bk_t)
        nc.gpsimd.dma_start(mk_sb[:], mk_t)

        # ---------------- Phase 1: K^T / V projections ----------------
        # (Q^T is computed on demand inside the attention loop so its PE work
        # overlaps the exp-bound steady state.)
        qwp = ctx.enter_context(tc.tile_pool(name="qwp", bufs=1))
        qw = [qwp.tile([P, E], F32R, name=f"qw{d}", tag=f"qw{d}") for d in range(NDCH)]
        with (
            tc.tile_pool(name="wpool", bufs=1) as wpool,
            tc.tile_pool(name="xpool", bufs=18) as xpool,
            tc.tile_pool(name="ppsum", bufs=3, space="PSUM") as ppsum,
        ):
            kw = [wpool.tile([P, E], F32R, name=f"kw{d}", tag=f"kw{d}") for d in range(NDCH)]
            vw = [wpool.tile([P, E], F32R, name=f"vw{d}", tag=f"vw{d}") for d in range(NDCH)]
            # weights go on the scalar-engine HWDGE queue (idle during P1) so
            # they stream in parallel with the x tiles on the sync queue.
            # vw/qw are emitted after sc0's x tiles (inside the loop) so they
            # don't delay the first K^T matmuls.
            for d in range(NDCH):
                nc.scalar.dma_start(kw[d][:, 0 : 2 * P], wkT[d * P : (d + 1) * P, 0 : 2 * P])
            for d in range(NDCH):
                nc.scalar.dma_start(kw[d][:, 2 * P :], wkT[d * P : (d + 1) * P, 2 * P :])

            for sc in range(NSC):
                ssl = slice(sc * QCW, (sc + 1) * QCW)
                xs = []
                for d in range(NDCH):
                    xt = xpool.tile([P, QCW], F32R, name="xt", tag="xt")
                    eng = nc.sync if d % 2 == 0 else nc.scalar
                    eng.dma_start(xt[:], xT[d * P : (d + 1) * P, ssl])
                    xs.append(xt)
                if sc == 0:
                    for d in range(NDCH):
                        nc.scalar.dma_start(vw[d][:], wvT[d * P : (d + 1) * P, :])
                    for d in range(NDCH):
                        nc.scalar.dma_start(qw[d][:], wqT[d * P : (d + 1) * P, :])

                # K^T / Q^T e-tiles: out[e(128), s(512)] = W^T-chunk.T @ x^T
                for W, bias_sb, OUT in ((kw, bk_sb, KT), (qw, bq_sb, QT)):
                    for j in range(NET):
                        ps = ppsum.tile([P, QCW], F32, name="pps", tag="pps")
                        for d in range(NDCH):
                            _mm(
                                nc,
                                ps[:],
                                W[d][:, j * P : (j + 1) * P],
                                xs[d][:],
                                start=(d == 0),
                                stop=(d == NDCH - 1),
                            )
                        nc.vector.tensor_scalar_add(
                            OUT[j][:, ssl], ps[:], bias_sb[:, j : j + 1]
                        )

                # V s-tiles: out[s(128), e(512)] = x^T-chunk.T @ Wv^T-chunk
                for t4 in range(4):
                    t = sc * 4 + t4
                    ps = ppsum.tile([P, QCW], F32, name="pps", tag="pps")
                    for d in range(NDCH):
                        _mm(
                            nc,
                            ps[:],
                            xs[d][:, t4 * P : (t4 + 1) * P],
                            vw[d][:],
                            start=(d == 0),
                            stop=(d == NDCH - 1),
                        )
                    vdst = Vg[t][:].rearrange("p (h c) -> p h c", c=VW)
                    nc.vector.tensor_scalar_mul(
                        vdst[:, :, 0:64],
                        ps[:].rearrange("p (h c) -> p h c", c=64),
                        mk_sb[:, t : t + 1],
                    )
                    nc.gpsimd.dma_start(vdst[:, :, 64], mask8[t])

        # ---------------- Phase 2: attention + output projection ----------------
        with (
            tc.tile_pool(name="wopool", bufs=1) as wopool,
            tc.tile_pool(name="ptpool", bufs=4) as ptpool,
            tc.tile_pool(name="otpool", bufs=2) as otpool,
            tc.tile_pool(name="npool", bufs=2) as npool,
            tc.tile_pool(name="ostage", bufs=3) as ostage,
            tc.tile_pool(name="spsum", bufs=2, space="PSUM") as spsum,
            tc.tile_pool(name="vpsum", bufs=2, space="PSUM") as vpsum,
        ):
            wo = [
                wopool.tile([P, D], F32R, name=f"wo{c}", tag=f"wo{c}") for c in range(4)
            ]
            for c in range(4):
                nc.scalar.dma_start(wo[c][:], woT[c * P : (c + 1) * P, :])

            for qc in range(NQC):
                qsl = slice(qc * QCW, (qc + 1) * QCW)
                OTs = [
                    otpool.tile([P, QCW], F32R, name=f"ot{hp}", tag=f"ot{hp}")
                    for hp in range(NHP)
                ]
                for hp in range(NHP):
                    pvA = vpsum.tile([P, QCW], F32, name="pv", tag="pv")
                    pvB = vpsum.tile([P, QCW], F32, name="pv", tag="pv")
                    pv_of = (pvA, pvB)

                    units = [(kc, h) for kc in range(NKC) for h in (0, 1)]
                    for g0 in range(0, len(units), EXP_GRP):
                        grp = units[g0 : g0 + EXP_GRP]
                        st = spsum.tile(
                            [P, QCW * EXP_GRP], F32, name="st", tag="st"
                        )
                        for i, (kc, h) in enumerate(grp):
                            lo = h * 64
                            _mm(
                                nc,
                                st[:, i * QCW : (i + 1) * QCW],
                                KT[hp][lo : lo + 64, kc * P : (kc + 1) * P],
                                QT[hp][lo : lo + 64, qsl],
                                start=True,
                                stop=True,
                            )
                        pt = ptpool.tile([P, QCW * EXP_GRP], F32R, name="pt", tag="pt")
                        nw = len(grp) * QCW
                        nc.scalar.activation(pt[:, :nw], st[:, :nw], AF.Exp, scale=0.125)
                        for i, (kc, h) in enumerate(grp):
                            hh = hp * 2 + h
                            _mm(
                                nc,
                                pv_of[h][0:VW, :],
                                Vg[kc][:, hh * VW : (hh + 1) * VW],
                                pt[:, i * QCW : (i + 1) * QCW],
                                start=(kc == 0),
                                stop=(kc == NKC - 1),
                            )

                    # softmax normalization; head A -> OT rows 0-63,
                    # head B -> OT rows 64-127 (via SBUF->SBUF DMA).
                    # The PV psum banks are evicted to SBUF immediately so the
                    # banks recycle fast; the rest of the chain runs off-path.
                    # HW quirks: partition_broadcast reads the source tile's
                    # physical partition 0 and writes from partition 0 only,
                    # so the reciprocal row is shifted to partition 0 first
                    # (single-input DVE copies may shift partition base).
                    pvsA = npool.tile([P, QCW], F32, name="pvsA", tag="pvsA")
                    pvsB = npool.tile([P, QCW], F32, name="pvsB", tag="pvsB")
                    rpA = npool.tile([P, QCW], F32, name="rpA", tag="rpA", bufs=1)
                    rpB = npool.tile([P, QCW], F32, name="rpB", tag="rpB", bufs=1)
                    rcA = npool.tile([P, QCW], F32, name="rcA", tag="rcA", bufs=1)
                    rcB = npool.tile([P, QCW], F32, name="rcB", tag="rcB", bufs=1)
                    bcA = npool.tile([P, QCW], F32, name="bcA", tag="bcA", bufs=1)
                    bcB = npool.tile([P, QCW], F32, name="bcB", tag="bcB", bufs=1)
                    tmB = npool.tile([P, QCW], F32R, name="tmB", tag="tmB")
                    nc.vector.tensor_copy(pvsA[0:VW, :], pvA[0:VW, :])
                    nc.vector.tensor_copy(pvsB[0:VW, :], pvB[0:VW, :])
                    # custom DVE ops misbehave off base partition 0 on HW:
                    # shift the sums row down first, then approx-recip at 0.
                    nc.vector.tensor_copy(rpA[0:1, :], pvsA[64:65, :])
                    nc.vector.tensor_copy(rpB[0:1, :], pvsB[64:65, :])
                    nc.vector.reciprocal_approx_fast(rcA[0:1, :], rpA[0:1, :])
                    nc.vector.reciprocal_approx_fast(rcB[0:1, :], rpB[0:1, :])
                    nc.gpsimd.partition_broadcast(bcA[0:64, :], rcA[0:1, :], channels=64)
                    nc.gpsimd.partition_broadcast(bcB[0:64, :], rcB[0:1, :], channels=64)
                    nc.vector.tensor_mul(OTs[hp][0:64, :], pvsA[0:64, :], bcA[0:64, :])
                    nc.vector.tensor_mul(tmB[0:64, :], pvsB[0:64, :], bcB[0:64, :])
                    nc.sync.dma_start(OTs[hp][64:128, :], tmB[0:64, :])

                # final^T[e, q] partial = sum_hp Wo^T-chunk.T @ O^T-chunk
                for j in range(D // P):
                    wops = vpsum.tile([P, QCW], F32, name="pv", tag="pv")
                    for hp in range(NHP):
                        _mm(
                            nc,
                            wops[:],
                            wo[hp][:, j * P : (j + 1) * P],
                            OTs[hp][:],
                            start=(hp == 0),
                            stop=(hp == NHP - 1),
                        )
                    ot = ostage.tile([P, QCW], F32, name="os", tag="os")
                    nc.vector.tensor_copy(ot[:], wops[:])
                    nc.sync.dma_start(fT[j * P : (j + 1) * P, qsl], ot[:])

    nc.compile()
    return nc


_PROGRAM = None


def _get_program():
    global _PROGRAM
    if _PROGRAM is None:
        _PROGRAM = _build_program()
    return _PROGRAM


def make_in_maps(x, mask, Wq, Wk, Wv, bq, bk):
    """Per-core input dicts. Core c: batch c//2, head-group c%2."""
    WqT = np.ascontiguousarray(Wq.T.astype(np.float32))
    WkT = np.ascontiguousarray(Wk.T.astype(np.float32))
    WvT = np.ascontiguousarray(Wv.T.astype(np.float32))
    in_maps = []
    for c in range(8):
        b, g = divmod(c, 2)
        esl = slice(g * E, (g + 1) * E)
        m = mask[b].astype(np.float32)
        mk = np.ascontiguousarray(m.reshape(NST, P).T)
        m8 = np.ascontiguousarray(
            np.repeat(m.reshape(NST, P, 1), NH, axis=2).astype(np.float32)
        )
        in_maps.append(
            {
                "xT": np.ascontiguousarray(x[b].T.astype(np.float32)),
                "wqT": np.ascontiguousarray(WqT[:, esl]),
                "wkT": np.ascontiguousarray(WkT[:, esl]),
                "wvT": np.ascontiguousarray(WvT[:, esl]),
                "bq_t": np.ascontiguousarray(bq[esl].reshape(NET, P).T.astype(np.float32)),
                "bk_t": np.ascontiguousarray(bk[esl].reshape(NET, P).T.astype(np.float32)),
                "mk_t": mk,
                "mask8": m8,
            }
        )
    return in_maps


def kernel(**inputs):
    x = np.asarray(inputs["x"], dtype=np.float32)
    mask = np.asarray(inputs["mask"])
    Wq = np.asarray(inputs["Wq"], dtype=np.float32)
    Wk = np.asarray(inputs["Wk"], dtype=np.float32)
    Wv = np.asarray(inputs["Wv"], dtype=np.float32)
    Wo = np.asarray(inputs["Wo"], dtype=np.float32)
    bq = np.asarray(inputs["bq"], dtype=np.float32)
    bk = np.asarray(inputs["bk"], dtype=np.float32)
    bv = np.asarray(inputs["bv"], dtype=np.float32)
    bo = np.asarray(inputs["bo"], dtype=np.float32)

    nc = _get_program()

    WoT = np.ascontiguousarray(Wo.T)  # [d, e]
    in_maps = make_in_maps(x, mask, Wq, Wk, Wv, bq, bk)
    for c in range(8):
        g = c % 2
        in_maps[c]["woT"] = np.ascontiguousarray(WoT[g * E : (g + 1) * E, :])

    res = run_bass_kernel_spmd(nc, in_maps, core_ids=list(range(8)))

    extra = (bv @ WoT + bo).astype(np.float32)  # [D]
    out = np.empty((4, S, D), dtype=np.float32)
    for b in range(4):
        acc = res.results[2 * b]["fT"] + res.results[2 * b + 1]["fT"]  # [D, S]
        out[b] = acc.T + extra[None, :]
    return out



# revision 14
# speedup vs baseline: 1.0045x; 1.0045x over previous
# Multi-head attention kernel for Trainium2 (Bass/Tile), 8-core SPMD.
#
# Problem: B=4, S=2048, D=1024, H=16 heads, d_k=64 (fp32 in/out).
#
# Sharding: core c = (batch b, head-group g) with b = c//2, g = c%2.
# Each core computes 8 heads of one batch entirely on-device and emits the
# partial final projection (out_heads @ Wo_slice^T) over the full model dim.
# Host sums the two partial outputs per batch and adds the linear bias terms.
#
# All matmul operands are bf16 (PE streams 1 elem/cycle for bf16 and fp32r
# alike, but bf16 halves DMA + SBUF traffic and enables fast weight loads);
# PSUM accumulation stays fp32.  Softmax exp runs on ScalarE out of PSUM —
# ScalarE is 1 elem/cycle/lane, so exp (~33.5M elems/core) is ~320us of
# ScalarE time and must overlap the PE stream everywhere.
#
# Schedule (single fused phase, engine-order = emission order):
#   ramp:   x (d,sc)-tiles stream in sc-major on three DMA queues while the
#           K projection consumes them sc by sc; then Q(qc0).  First score
#           group ~35us in.
#   beats:  per group g of 3 (h,kc) units: scores(g) -> exp(g) -> attnV(g-1)
#           -> one background item.  attnV lags one group so the PE never
#           waits on ScalarE; h-sequential attnV keeps one PSUM bank.
#   qc0/hp0: attnV deferred for the whole head-pair while the 16 V-projection
#           tiles run as background work under the first exp stream.
#   background queue (one shared PSUM bank): V tiles, Q(qc+1) projections,
#           outproj(qc-1) — keeps the PE dense and HAM warm.
#   PSUM budget: scores 2x3 banks + attn-V 1 + background 1 = 8.
#
# The V tiles carry the key mask folded in, plus the mask itself as a 65th
# column per head so the attention-V matmul also produces the softmax
# denominators (row 64).  Normalization: reciprocal of row 64, gpsimd
# partition broadcast, one DVE multiply per head.
#
# Biases: bq/bk added on device (folded into PSUM eviction). bv and bo are
# linear post-softmax terms: since softmax rows sum to one,
# (attn@V + bv)@Wo^T + bo == attn@V@Wo^T + (bv@Wo^T + bo), added on host.

from collections import deque
from contextlib import ExitStack

import numpy as np
import ml_dtypes

import concourse.bass as bass  # noqa: F401  (AP types come via handles)
import concourse.tile as tile
from concourse import bacc, mybir
from concourse.bass_utils import run_bass_kernel_spmd

P = 128
S = 2048          # sequence length
D = 1024          # model dim
E = 512           # per-core head dims (8 heads x 64)
NH = 8            # heads per core
NDCH = D // P     # 8 contraction chunks for projections
NST = S // P      # 16 s-tiles (key chunks)
NSC = 4           # s chunks of 512
NET = E // P      # 4 e-tiles of the local head dims
NHP = NH // 2     # 4 head pairs
NKC = NST         # 16 key chunks of 128
NQC = 4           # query chunks of 512
QCW = S // NQC    # 512
VW = 65           # V columns per head incl. mask column

F32 = mybir.dt.float32
BF16 = mybir.dt.bfloat16
AF = mybir.ActivationFunctionType

EXP_GRP = 3       # scores tiles per exp instruction (3 PSUM banks)


def _mm(nc, out, lhsT, rhs, start, stop):
    nc.tensor.matmul(out, lhsT, rhs, start=start, stop=stop)


def _build_program():
    nc = bacc.Bacc(
        "TRN2",
        debug=False,
        target_bir_lowering=False,
        enable_partition_id=False,
    )

    xT = nc.dram_tensor("xT", [D, S], BF16, kind="ExternalInput").ap()
    wqT = nc.dram_tensor("wqT", [D, E], BF16, kind="ExternalInput").ap()
    wkT = nc.dram_tensor("wkT", [D, E], BF16, kind="ExternalInput").ap()
    wvT = nc.dram_tensor("wvT", [D, E], BF16, kind="ExternalInput").ap()
    woT = nc.dram_tensor("woT", [E, D], BF16, kind="ExternalInput").ap()
    bq_t = nc.dram_tensor("bq_t", [P, NET], F32, kind="ExternalInput").ap()
    bk_t = nc.dram_tensor("bk_t", [P, NET], F32, kind="ExternalInput").ap()
    mk_t = nc.dram_tensor("mk_t", [P, NST], F32, kind="ExternalInput").ap()
    mask8 = nc.dram_tensor("mask8", [NST, P, NH], BF16, kind="ExternalInput").ap()
    fT = nc.dram_tensor("fT", [D, S], F32, kind="ExternalOutput").ap()

    with tile.TileContext(nc) as tc, ExitStack() as ctx:
        pers = ctx.enter_context(tc.tile_pool(name="pers", bufs=1))

        KT = [pers.tile([P, S], BF16, name=f"KT{j}", tag=f"KT{j}") for j in range(NET)]
        QT = [pers.tile([P, S], BF16, name=f"QT{j}", tag=f"QT{j}") for j in range(NET)]
        Vg = [
            pers.tile([P, NH * VW], BF16, name=f"Vg{t}", tag=f"Vg{t}")
            for t in range(NST)
        ]
        xs = [
            [pers.tile([P, QCW], BF16, name=f"x{d}_{sc}", tag=f"x{d}_{sc}")
             for sc in range(NSC)]
            for d in range(NDCH)
        ]
        qw = [pers.tile([P, E], BF16, name=f"qw{d}", tag=f"qw{d}") for d in range(NDCH)]
        wo = [pers.tile([P, D], BF16, name=f"wo{c}", tag=f"wo{c}") for c in range(NET)]
        bq_sb = pers.tile([P, NET], F32, name="bq_sb", tag="bq_sb")
        bk_sb = pers.tile([P, NET], F32, name="bk_sb", tag="bk_sb")
        mk_sb = pers.tile([P, NST], F32, name="mk_sb", tag="mk_sb")
        nc.gpsimd.dma_start(bq_sb[:], bq_t)
        nc.gpsimd.dma_start(bk_sb[:], bk_t)
        nc.gpsimd.dma_start(mk_sb[:], mk_t)

        wpool = ctx.enter_context(tc.tile_pool(name="wpool", bufs=1))
        kw = [wpool.tile([P, E], BF16, name=f"kw{d}", tag=f"kw{d}") for d in range(NDCH)]
        vw = [wpool.tile([P, E], BF16, name=f"vw{d}", tag=f"vw{d}") for d in range(NDCH)]
        wu = wpool.tile([P, P], BF16, name="wu", tag="wu")
        nc.gpsimd.memset(wu[:], 0.0)

        # x tiles sc-major across three DMA queues; weights on the scalar
        # queue (K weights first so the ramp projections start immediately).
        xq = [nc.sync, nc.gpsimd]
        qi = 0
        for sc in range(NSC):
            for d in range(NDCH):
                xq[qi % 2].dma_start(
                    xs[d][sc][:], xT[d * P : (d + 1) * P, sc * QCW : (sc + 1) * QCW]
                )
                qi += 1
        for d in range(NDCH):
            nc.scalar.dma_start(kw[d][:], wkT[d * P : (d + 1) * P, :])
        for d in range(NDCH):
            nc.scalar.dma_start(qw[d][:], wqT[d * P : (d + 1) * P, :])
        for d in range(NDCH):
            nc.scalar.dma_start(vw[d][:], wvT[d * P : (d + 1) * P, :])
        for c in range(NET):
            nc.scalar.dma_start(wo[c][:], woT[c * P : (c + 1) * P, :])

        # ---------------- ramp: K + Q(qc0) projections ----------------
        with tc.tile_pool(name="rampp", bufs=3, space="PSUM") as rampp:
            # PE warm-up burst: ~6us of dummy matmuls so the HAM clock gate
            # opens (K=8/8) before the real projections start, and the PE
            # isn't idle while the first x tiles stream in.
            wps = rampp.tile([P, 64], F32, name="wps", tag="wps")
            for i in range(120):
                _mm(nc, wps[:64, :], wu[:, (i % 2) * 64 : (i % 2) * 64 + 64],
                    wu[:, 64:128], start=(i == 0), stop=(i == 119))
            for sc in range(NSC):
                ssl = slice(sc * QCW, (sc + 1) * QCW)
                for j in range(NET):
                    ps = rampp.tile([P, QCW], F32, name="rps", tag="rps")
                    for d in range(NDCH):
                        _mm(nc, ps[:], kw[d][:, j * P : (j + 1) * P],
                            xs[d][sc][:], start=(d == 0), stop=(d == NDCH - 1))
                    nc.vector.tensor_scalar_add(
                        KT[j][:, ssl], ps[:], bk_sb[:, j : j + 1]
                    )
            for j in range(NET):
                ps = rampp.tile([P, QCW], F32, name="rps", tag="rps")
                for d in range(NDCH):
                    _mm(nc, ps[:], qw[d][:, j * P : (j + 1) * P],
                        xs[d][0][:], start=(d == 0), stop=(d == NDCH - 1))
                nc.vector.tensor_scalar_add(
                    QT[j][:, 0:QCW], ps[:], bq_sb[:, j : j + 1]
                )

        # ---------------- fused attention + background work ----------------
        with (
            tc.tile_pool(name="spsum", bufs=2, space="PSUM") as spsum,
            tc.tile_pool(name="pvpsum", bufs=1, space="PSUM") as pvpsum,
            tc.tile_pool(name="bgpsum", bufs=1, space="PSUM") as bgpsum,
            tc.tile_pool(name="ptpool", bufs=15) as ptpool,
            tc.tile_pool(name="otpool", bufs=2) as otpool,
            tc.tile_pool(name="npool", bufs=2) as npool,
            tc.tile_pool(name="ostage", bufs=3) as ostage,
        ):
            # --- background emitters (one PSUM bank through bgpsum) ---
            def bg_v(t):
                def emit():
                    ps = bgpsum.tile([P, QCW], F32, name="bgps", tag="bgps")
                    for d in range(NDCH):
                        _mm(nc, ps[:],
                            xs[d][t // 4][:, (t % 4) * P : (t % 4 + 1) * P],
                            vw[d][:], start=(d == 0), stop=(d == NDCH - 1))
                    vdst = Vg[t][:].rearrange("p (h c) -> p h c", c=VW)
                    nc.vector.tensor_scalar_mul(
                        vdst[:, :, 0:64],
                        ps[:].rearrange("p (h c) -> p h c", c=64),
                        mk_sb[:, t : t + 1],
                    )
                    nc.gpsimd.dma_start(vdst[:, :, 64], mask8[t])
                return emit

            def bg_q(qc, j):
                def emit():
                    ps = bgpsum.tile([P, QCW], F32, name="bgps", tag="bgps")
                    for d in range(NDCH):
                        _mm(nc, ps[:], qw[d][:, j * P : (j + 1) * P],
                            xs[d][qc][:], start=(d == 0), stop=(d == NDCH - 1))
                    nc.vector.tensor_scalar_add(
                        QT[j][:, qc * QCW : (qc + 1) * QCW], ps[:], bq_sb[:, j : j + 1]
                    )
                return emit

            def bg_outproj(qc, j, OTs, pool=None):
                qsl = slice(qc * QCW, (qc + 1) * QCW)

                def emit():
                    pl = pool or bgpsum
                    tg = "pv" if pl is pvpsum else "bgps"
                    ps = pl.tile([P, QCW], F32, name="bgps", tag=tg)
                    for hp in range(NHP):
                        _mm(nc, ps[:], wo[hp][:, j * P : (j + 1) * P], OTs[hp][:],
                            start=(hp == 0), stop=(hp == NHP - 1))
                    ot = ostage.tile([P, QCW], F32, name="os", tag="os")
                    nc.vector.tensor_copy(ot[:], ps[:])
                    nc.sync.dma_start(fT[j * P : (j + 1) * P, qsl], ot[:])
                return emit

            bg = deque(bg_v(t) for t in range(NST))

            def pump(n):
                for _ in range(n):
                    if bg:
                        bg.popleft()()

            def norm_head(pv, h, OT):
                # evict PSUM fast, then off-path normalization chain.
                # HW quirks: partition_broadcast reads physical partition 0
                # and single-input DVE copies may shift partition base, so
                # the sums row is copied down to partition 0 first.
                pvs = npool.tile([P, QCW], F32, name=f"pvs{h}", tag=f"pvs{h}")
                rp = npool.tile([P, QCW], F32, name="rp", tag="rp", bufs=1)
                rc = npool.tile([P, QCW], F32, name="rc", tag="rc", bufs=1)
                bc = npool.tile([P, QCW], F32, name=f"bc{h}", tag=f"bc{h}", bufs=1)
                nc.vector.tensor_copy(pvs[0:VW, :], pv[0:VW, :])
                nc.vector.tensor_copy(rp[0:1, :], pvs[64:65, :])
                nc.vector.reciprocal_approx_fast(rc[0:1, :], rp[0:1, :])
                nc.gpsimd.partition_broadcast(bc[0:64, :], rc[0:1, :], channels=64)
                if h == 0:
                    nc.vector.tensor_mul(OT[0:64, :], pvs[0:64, :], bc[0:64, :])
                else:
                    tmB = npool.tile([P, QCW], BF16, name="tmB", tag="tmB")
                    nc.vector.tensor_mul(tmB[0:64, :], pvs[0:64, :], bc[0:64, :])
                    nc.sync.dma_start(OT[64:128, :], tmB[0:64, :])

            # pending attn-V groups: (hp, units, pt, OT) — emitted two groups
            # behind the exp stream so the PE never waits on ScalarE (exp of
            # group g-2 is guaranteed complete by the time the PE reaches
            # attn-V(g-2) in its queue).
            pending = deque()
            pv_state = {"pv": None, "key": None}

            def flush_one():
                hp, units, pt, OT = pending.popleft()
                for (h, kc, i) in units:
                    if pv_state["key"] != (hp, h):
                        pv_state["pv"] = pvpsum.tile([P, QCW], F32, name="pv", tag="pv")
                        pv_state["key"] = (hp, h)
                    pv = pv_state["pv"]
                    hh = hp * 2 + h
                    _mm(
                        nc,
                        pv[0:VW, :],
                        Vg[kc][:, hh * VW : (hh + 1) * VW],
                        pt[:, i * QCW : (i + 1) * QCW],
                        start=(kc == 0),
                        stop=(kc == NKC - 1),
                    )
                    if kc == NKC - 1:
                        norm_head(pv, h, OT)

            prev_OTs = None
            for qc in range(NQC):
                qsl = slice(qc * QCW, (qc + 1) * QCW)
                OTs = [
                    otpool.tile([P, QCW], BF16, name=f"ot{hp}", tag=f"ot{hp}")
                    for hp in range(NHP)
                ]
                if qc + 1 < NQC:
                    for j in range(NET):
                        bg.append(bg_q(qc + 1, j))
                if prev_OTs is not None:
                    for j in range(D // P):
                        bg.append(bg_outproj(qc - 1, j, prev_OTs))

                for hp in range(NHP):
                    defer_all = qc == 0 and hp == 0
                    # per-h groups: 16 kc = 3+3+3+3+3+1, 12 exp instructions
                    # per head pair.
                    groups = [
                        [(h, kc) for kc in range(g0, min(g0 + EXP_GRP, NKC))]
                        for h in (0, 1)
                        for g0 in range(0, NKC, EXP_GRP)
                    ]
                    for grp in groups:
                        st = spsum.tile([P, QCW * EXP_GRP], F32, name="st", tag="st")
                        for i, (h, kc) in enumerate(grp):
                            lo = h * 64
                            _mm(
                                nc,
                                st[:, i * QCW : (i + 1) * QCW],
                                KT[hp][lo : lo + 64, kc * P : (kc + 1) * P],
                                QT[hp][lo : lo + 64, qsl],
                                start=True,
                                stop=True,
                            )
                        pt = ptpool.tile([P, QCW * EXP_GRP], BF16, name="pt", tag="pt")
                        nw = len(grp) * QCW
                        nc.scalar.activation(pt[:, :nw], st[:, :nw], AF.Exp, scale=0.125)
                        pending.append(
                            (hp, [(h, kc, i) for i, (h, kc) in enumerate(grp)], pt, OTs[hp])
                        )
                        if defer_all:
                            # V projections run as background work under the
                            # first exp stream; attn-V for this head pair
                            # drains gradually during hp1.
                            pump(2)
                        else:
                            flushed = 0
                            while len(pending) > 2 and flushed < 2:
                                flush_one()
                                flushed += 1
                            pump(1)
                prev_OTs = OTs

            while pending:
                flush_one()
            # tail: outproj for the last qc, alternating between the two free
            # PSUM pools so the evictions pipeline.
            for j in range(D // P):
                bg_outproj(NQC - 1, j, prev_OTs, pvpsum if j % 2 else bgpsum)()

    nc.compile()
    return nc


_PROGRAM = None


def _get_program():
    global _PROGRAM
    if _PROGRAM is None:
        _PROGRAM = _build_program()
    return _PROGRAM


def make_in_maps(x, mask, Wq, Wk, Wv, Wo, bq, bk):
    """Per-core input dicts. Core c: batch c//2, head-group c%2."""
    bf = ml_dtypes.bfloat16
    WqT = np.ascontiguousarray(Wq.T.astype(bf))
    WkT = np.ascontiguousarray(Wk.T.astype(bf))
    WvT = np.ascontiguousarray(Wv.T.astype(bf))
    WoT = np.ascontiguousarray(Wo.T.astype(np.float32))  # [d, e]
    in_maps = []
    for c in range(8):
        b, g = divmod(c, 2)
        esl = slice(g * E, (g + 1) * E)
        m = mask[b].astype(np.float32)
        mk = np.ascontiguousarray(m.reshape(NST, P).T)
        m8 = np.ascontiguousarray(
            np.repeat(m.reshape(NST, P, 1), NH, axis=2).astype(bf)
        )
        in_maps.append(
            {
                "xT": np.ascontiguousarray(x[b].T.astype(bf)),
                "wqT": np.ascontiguousarray(WqT[:, esl]),
                "wkT": np.ascontiguousarray(WkT[:, esl]),
                "wvT": np.ascontiguousarray(WvT[:, esl]),
                "woT": np.ascontiguousarray(WoT[esl, :].astype(bf)),
                "bq_t": np.ascontiguousarray(bq[esl].reshape(NET, P).T.astype(np.float32)),
                "bk_t": np.ascontiguousarray(bk[esl].reshape(NET, P).T.astype(np.float32)),
                "mk_t": mk,
                "mask8": m8,
            }
        )
    return in_maps


def kernel(**inputs):
    x = np.asarray(inputs["x"], dtype=np.float32)
    mask = np.asarray(inputs["mask"])
    Wq = np.asarray(inputs["Wq"], dtype=np.float32)
    Wk = np.asarray(inputs["Wk"], dtype=np.float32)
    Wv = np.asarray(inputs["Wv"], dtype=np.float32)
    Wo = np.asarray(inputs["Wo"], dtype=np.float32)
    bq = np.asarray(inputs["bq"], dtype=np.float32)
    bk = np.asarray(inputs["bk"], dtype=np.float32)
    bv = np.asarray(inputs["bv"], dtype=np.float32)
    bo = np.asarray(inputs["bo"], dtype=np.float32)

    nc = _get_program()
    in_maps = make_in_maps(x, mask, Wq, Wk, Wv, Wo, bq, bk)

    res = run_bass_kernel_spmd(nc, in_maps, core_ids=list(range(8)))

    WoT = Wo.T  # [d, e]
    extra = (bv @ WoT + bo).astype(np.float32)  # [D]
    out = np.empty((4, S, D), dtype=np.float32)
    for b in range(4):
        acc = res.results[2 * b]["fT"] + res.results[2 * b + 1]["fT"]  # [D, S]
        out[b] = acc.T + extra[None, :]
    return out
